# revision 27
# baseline (speedup 1.0000x reference)
"""MultiHeadAttention Trainium2 Bass kernel.

Problem: B=4, S=2048, C=512, H=8, D=64 MHA with learned relative-position
bias table gathered by bias_idxs == ones(49,49).  That gather makes the
bias a per-head constant, which is invariant under softmax over the key
axis, so the bias path is mathematically a no-op and is dropped.

Sharding (8 cores): core c handles batch b = c//2 and head-group
g = c%2 (4 heads = 256 channels).  Wq/Wk/Wv are sharded on their output
dim, Wo on its input dim; the two head-group partial outputs per batch
are summed on the host (the post-projection all-reduce).

Per-core device program (matmul operands bf16, scores bf16 PSUM):
  qT,kT = (x Wq^T)^T etc. as [256, 2048] channel-major tiles
  v     = x Wv^T as [2048, 256] token-major (+ ones column per head)
  attention per (query-chunk, head-pair): heads 2p / 2p+1 live at
  partition bases 0 / 64, so their K=64 score matmuls occupy disjoint
  PE row-groups; scores for a 2-key-chunk unit land in one bf16 psum
  tile (2 banks) so exp covers 2048 elems/lane per ACT instruction.
  PV uses a ones-column (M=65) so the softmax denominator Z comes out
  as psum row 64 for free.  The Z/normalize tail is split per pair and
  the output projection accumulates pair 0's half mid-loop, so only the
  final pair's short tail trails the exp stream.
"""

import numpy as np
import ml_dtypes

P = 128
S = 2048          # sequence
CIN = 512         # model dim
CG = 256          # channels per head-group (4 heads x 64)
D = 64            # head dim
NH = 4            # heads per group
QC = 512          # query chunk (psum bank)
NQC = S // QC     # 4
NKC = S // P      # 16 key chunks of 128
NU = NKC // 2     # 8 units of 2 key chunks per (qc, pair)

_CACHE = {}


def _build_nc(loop_n=1):
    import contextlib
    import concourse.tile as tile
    from concourse import bacc, mybir

    bf16 = mybir.dt.bfloat16
    f16 = mybir.dt.float16
    f32 = mybir.dt.float32

    nc = bacc.Bacc("TRN2", target_bir_lowering=False, debug=False, num_devices=8)

    xT = nc.dram_tensor("xT", [CIN, S], bf16, kind="ExternalInput")
    wqT = nc.dram_tensor("wqT", [CIN, CG], bf16, kind="ExternalInput")
    wkT = nc.dram_tensor("wkT", [CIN, CG], bf16, kind="ExternalInput")
    wvT = nc.dram_tensor("wvT", [CIN, CG], bf16, kind="ExternalInput")
    woT = nc.dram_tensor("woT", [CG, CIN], bf16, kind="ExternalInput")
    bq = nc.dram_tensor("bq", [CG], f32, kind="ExternalInput")
    bk = nc.dram_tensor("bk", [CG], f32, kind="ExternalInput")
    bv = nc.dram_tensor("bv", [CG], f32, kind="ExternalInput")
    bo = nc.dram_tensor("bo", [CIN], f32, kind="ExternalInput")
    outT = nc.dram_tensor("outT", [CIN, S], bf16, kind="ExternalOutput")

    # exp chunks routed to the DVE (squaring-polynomial approximation) to
    # offload the saturated ACT engine: skip (qc 0, pair 0) where the DVE
    # is busy with the woven projection copies.
    # (measured in CoreSim: the 5-op DVE chain's latency on a loaded FIFO
    # stalls the PV accumulation; net loss -> disabled)
    dve_exp = set()

    with tile.TileContext(nc) as tc:
        # bench-only: repeat the whole body on-device to amplify exec time
        # above the PJRT dispatch noise floor
        loop_cm = tc.For_i(0, loop_n, 1) if loop_n > 1 else contextlib.nullcontext()
        with loop_cm, \
             tc.tile_pool(name="const", bufs=1) as const, \
             tc.tile_pool(name="big", bufs=1) as big, \
             tc.tile_pool(name="pt", bufs=4) as ptp, \
             tc.tile_pool(name="zs", bufs=4) as zsp, \
             tc.tile_pool(name="rzstage", bufs=2) as rzsp, \
             tc.tile_pool(name="spool", bufs=2, space="PSUM") as sp, \
             tc.tile_pool(name="tailp", bufs=1, space="PSUM") as tp, \
             tc.tile_pool(name="pvpool", bufs=3, space="PSUM") as pvp:

            def pv_tile(name="pvt"):
                # shared slots: PV accumulators, projection psum, all
                # [P, 512] fp32 = 1 bank
                return pvp.tile([P, QC], f32, tag="pv", name=name)

            def tp_tile():
                # tail-path psum (zstack / recip-broadcast / out-proj)
                return tp.tile([P, 1, QC], f32, tag="t", name="tpt")

            # ---------- PE warm-up ----------
            # dummy matmul chain during the input-DMA wait: keeps the PE
            # busy from t=0 so the HAM clock gate is already at full rate
            # (2.4 GHz) when the first projection matmuls arrive
            warm_sb = const.tile([1, QC], bf16, tag="warm")
            nc.vector.memset(warm_sb[:], 0.0)
            warm_ps = tp_tile()
            for _ in range(18):
                nc.tensor.matmul(
                    warm_ps[:1, 0, :], warm_sb[:, :1], warm_sb[:, :],
                    start=True, stop=True,
                )

            # ---------- load inputs ----------
            # k/q weights + x chunk 0 first so the first projections start
            # as early as possible; everything else streams behind them.
            wk_sb = big.tile([P, CIN // P, CG], bf16, tag="wk")
            nc.sync.dma_start(wk_sb[:], wkT.rearrange("(o p) c -> p o c", p=P))
            wq_sb = big.tile([P, CIN // P, CG], bf16, tag="wq")
            nc.sync.dma_start(wq_sb[:], wqT.rearrange("(o p) c -> p o c", p=P))
            xT_sb = big.tile([P, CIN // P, S], bf16, tag="xT")
            xT_r = xT.rearrange("(o p) t -> p o t", p=P)
            nc.sync.dma_start(xT_sb[:, :, 0:QC // 2], xT_r[:, :, 0:QC // 2])
            nc.sync.dma_start(xT_sb[:, :, QC // 2:QC], xT_r[:, :, QC // 2:QC])
            bk_sb = const.tile([P, CG // P], f32, tag="bk")
            nc.sync.dma_start(bk_sb[:], bk.rearrange("(s p) -> p s", p=P))
            bq_sb = const.tile([P, CG // P], f32, tag="bq")
            nc.sync.dma_start(bq_sb[:], bq.rearrange("(s p) -> p s", p=P))
            wv_sb = big.tile([P, CIN // P, CG], bf16, tag="wv")
            nc.sync.dma_start(wv_sb[:], wvT.rearrange("(o p) c -> p o c", p=P))
            bv_sb = const.tile([P, CG // P], f32, tag="bv")
            nc.sync.dma_start(bv_sb[:], bv.rearrange("(s p) -> p s", p=P))
            for t in range(1, NQC):
                tsl = slice(t * QC, (t + 1) * QC)
                nc.sync.dma_start(xT_sb[:, :, tsl], xT_r[:, :, tsl])
            wo_sb = big.tile([P, CG // P, CIN], bf16, tag="wo")
            nc.sync.dma_start(wo_sb[:], woT.rearrange("(o p) c -> p o c", p=P))
            bo_sb = const.tile([P, CIN // P], f32, tag="bo")
            nc.sync.dma_start(bo_sb[:], bo.rearrange("(s p) -> p s", p=P))
            # all-ones [1, 128] selector: broadcasts a [1, N] row across
            # all partitions via a K=1 matmul
            ones_sb = const.tile([1, P], f16, tag="ones1")
            nc.vector.memset(ones_sb[:], 1.0)

            # ---------- projections ----------
            qT_sb = big.tile([P, CG // P, S], bf16, tag="qT")
            kT_sb = big.tile([P, CG // P, S], bf16, tag="kT")
            # v token-major with a ones column per head (for Z)
            v_sb = big.tile([P, NKC, NH, D + 1], f16, tag="v")
            nc.vector.memset(v_sb[:], 1.0)

            # channel-major qT/kT projection for one (cout-slice, token range)
            def proj_qk_rng(dst, w, b, s, t0, t1):
                pj = pv_tile(name="pjqk")
                for ci in range(CIN // P):
                    nc.tensor.matmul(
                        pj[:, :t1 - t0],
                        w[:, ci, s * P:(s + 1) * P],
                        xT_sb[:, ci, t0:t1],
                        start=(ci == 0),
                        stop=(ci == CIN // P - 1),
                    )
                nc.vector.tensor_scalar_add(
                    dst[:, s, t0:t1], pj[:, :t1 - t0], b[:, s:s + 1],
                )

            def proj_qk(dst, w, b, s, t):
                proj_qk_rng(dst, w, b, s, t * QC, (t + 1) * QC)

            # token-major v for one 128-token slice; bv is applied after
            # normalization (probs sum to 1, so ctx/Z + bv is exact)
            def proj_v(t):
                pj = pv_tile(name="pjv")
                for ci in range(CIN // P):
                    nc.tensor.matmul(
                        pj[:, :CG],
                        xT_sb[:, ci, t * P:(t + 1) * P],
                        wv_sb[:, ci, :],
                        start=(ci == 0),
                        stop=(ci == CIN // P - 1),
                    )
                nc.vector.tensor_copy(
                    v_sb[:, t, :, :D],
                    pj[:, :CG].rearrange("p (h d) -> p h d", d=D),
                )

            # Emit the minimal projection prefix needed for the first score
            # matmuls, then weave the rest into the qc-0 attention loop.
            # kT for the first two key chunks comes off the first half-DMA
            # of x so the score/exp stream starts as early as possible.
            for s in range(CG // P):
                proj_qk_rng(kT_sb, wk_sb, bk_sb, s, 0, QC // 2)
            for s in range(CG // P):
                proj_qk(qT_sb, wq_sb, bq_sb, s, 0)
            for s in range(CG // P):
                proj_qk_rng(kT_sb, wk_sb, bk_sb, s, QC // 2, QC)
            proj_v(0)
            proj_v(1)
            # remaining work queue, consumed inside attention qc 0:
            # before unit u (kc 2u, 2u+1) we need kT t <= (2u+1)//4 and
            # v kc <= 2u+1 (plus lookahead).
            pending = []
            for t in range(1, NQC):
                pending.append(("kq", t))
            for t in range(2, NKC):
                pending.append(("v", t))

            def emit_proj_upto(u):
                need_kt = (2 * u + 1) // 4
                need_v = 2 * u + 1
                for item in list(pending):
                    kind, t = item
                    if kind == "kq" and t <= need_kt + 1:
                        for s in range(CG // P):
                            proj_qk(kT_sb, wk_sb, bk_sb, s, t)
                    elif kind == "v" and t <= need_v + 2:
                        proj_v(t)
                    else:
                        continue
                    pending.remove(item)

            def emit_proj_rest():
                for kind, t in pending:
                    if kind == "kq":
                        for s in range(CG // P):
                            proj_qk(kT_sb, wk_sb, bk_sb, s, t)
                    else:
                        proj_v(t)
                pending.clear()

            # ---------- attention ----------
            ctx_raw = big.tile([P, CG // P, S], bf16, tag="ctxr")
            ctx_nrm = big.tile([P, CG // P, S], bf16, tag="ctxn")
            outT_sb = big.tile([P, CIN // P, S], bf16, tag="outT")
            outT_r = outT.rearrange("(o p) t -> p o t", p=P)

            for qc in range(NQC):
                qsl = slice(qc * QC, (qc + 1) * QC)
                for pair in range(2):
                    pvs = [pv_tile(name=f"pv{i}") for i in range(2)]
                    for kc in range(NKC):
                        # scores for 1 kc x 2 heads into one fp32 psum tile
                        # (2 banks); heads 2p/2p+1 sit at partition bases
                        # 0/64 -> disjoint PE row-groups.
                        st = sp.tile([P, 2, QC], f32, tag="s", name="spt")
                        for i in range(2):
                            h = 2 * pair + i
                            hp, hs = D * (h % 2), h // 2
                            nc.tensor.matmul(
                                st[:, i, :],
                                kT_sb[hp:hp + D, hs, kc * P:(kc + 1) * P],
                                qT_sb[hp:hp + D, hs, qsl],
                                start=True, stop=True,
                                tile_position=(hp, 0),
                            )
                        pt = ptp.tile([P, 2, QC], f16, tag="pt", name="pt")
                        if (qc, pair, kc) in dve_exp:
                            # exp(s/8) = ((1 + h + h^2/2)^2)^2 with h = s/32:
                            # |h| <= ~0.3 so the truncation error is tiny;
                            # fp16 keeps 1+h to ~5e-4.  Offloads the
                            # saturated ACT engine onto DVE slack.
                            u_t = zsp.tile([P, 2, QC], f16, tag="eu", name="eu")
                            nc.vector.tensor_scalar(
                                u_t[:], st[:], 1.0 / 32.0, 1.0,
                                mybir.AluOpType.mult, mybir.AluOpType.add)
                            w_t = zsp.tile([P, 2, QC], f16, tag="ew", name="ew")
                            nc.vector.tensor_tensor(
                                w_t[:], u_t[:], u_t[:], mybir.AluOpType.mult)
                            nc.vector.tensor_scalar(
                                u_t[:], w_t[:], 0.5, 0.5,
                                mybir.AluOpType.mult, mybir.AluOpType.add)
                            nc.vector.tensor_tensor(
                                w_t[:], u_t[:], u_t[:], mybir.AluOpType.mult)
                            nc.vector.tensor_tensor(
                                pt[:], w_t[:], w_t[:], mybir.AluOpType.mult)
                        else:
                            nc.scalar.activation(
                                pt[:], st[:],
                                mybir.ActivationFunctionType.Exp,
                                bias=0.0, scale=0.125,
                            )
                        for i in range(2):
                            h = 2 * pair + i
                            nc.tensor.matmul(
                                pvs[i][:D + 1, :],
                                v_sb[:, kc, h, :],
                                pt[:, i, :],
                                start=(kc == 0),
                                stop=(kc == NKC - 1),
                            )
                        if qc == 0 and pair == 0 and kc % 2 == 1:
                            emit_proj_upto(kc // 2 + 1)
                        if qc < NQC - 1 and pair == 1 and kc == 5:
                            # prefetch next qc's queries mid-loop
                            for s in range(CG // P):
                                proj_qk(qT_sb, wq_sb, bq_sb, s, qc + 1)
                    if qc == 0 and pair == 0:
                        emit_proj_rest()

                    # ---- per-pair tail: ctx stash + Z + normalize ----
                    # the very last tail runs after the exp stream is done,
                    # so route its copies through the then-idle ACT engine
                    # to shorten the serial epilogue chain
                    last_tail = (qc == NQC - 1 and pair == 1)
                    cp = nc.scalar.copy if last_tail else nc.vector.tensor_copy
                    # 1/Z straight off the psum Z rows (row 64 of each PV
                    # accumulator); broadcast across partitions below with an
                    # all-ones K=1 matmul
                    rzs = []
                    for i in range(2):
                        h = 2 * pair + i
                        hp, hs = D * (h % 2), h // 2
                        cp(ctx_raw[hp:hp + D, hs, qsl], pvs[i][:D, :])
                        rz = rzsp.tile([1, QC], f16, tag="rz")
                        with nc.allow_low_precision(
                                reason="1/Z in fp16: Z ~ O(2048), step 2^-11"):
                            nc.vector.reciprocal(rz[:], pvs[i][D:D + 1, :])
                        rzs.append(rz)
                    for i in range(2):
                        h = 2 * pair + i
                        hp, hs = D * (h % 2), h // 2
                        bc = tp_tile()
                        nc.tensor.matmul(
                            bc[:, 0, :],
                            ones_sb[:, :],
                            rzs[i][:],
                            start=True, stop=True,
                        )
                        sl = (slice(hp, hp + D), hs, qsl)
                        nc.vector.tensor_tensor(
                            ctx_nrm[sl], ctx_raw[sl], bc[hp:hp + D, 0, :],
                            mybir.AluOpType.mult,
                        )
                        nc.vector.tensor_scalar_add(
                            ctx_nrm[sl], ctx_nrm[sl], bv_sb[hp:hp + D, hs:hs + 1],
                        )

                # ---- output projection (pair-1 tail; both ctx halves are
                # ready since pair 0's normalize ran mid-loop) ----
                for oc in range(CIN // P):
                    if qc == NQC - 1:
                        # pv slots are free in the last qc's tail; using
                        # them (bufs=3) instead of the single tail bank lets
                        # the MM -> bias-add -> DMA chains of consecutive oc
                        # overlap.  Mid-loop they'd steal the next pair's PV
                        # accumulators, so only the final tail does this.
                        op = pv_tile(name="op")[:, :].rearrange(
                            "p (o q) -> p o q", o=1)
                    else:
                        op = tp_tile()
                    for s in range(CG // P):
                        nc.tensor.matmul(
                            op[:, 0, :],
                            wo_sb[:, s, oc * P:(oc + 1) * P],
                            ctx_nrm[:, s, qsl],
                            start=(s == 0),
                            stop=(s == CG // P - 1),
                        )
                    if qc == NQC - 1:
                        nc.scalar.activation(
                            outT_sb[:, oc, qsl], op[:, 0, :],
                            mybir.ActivationFunctionType.Identity,
                            bias=bo_sb[:, oc:oc + 1], scale=1.0,
                        )
                    else:
                        nc.vector.tensor_scalar_add(
                            outT_sb[:, oc, qsl], op[:, 0, :], bo_sb[:, oc:oc + 1],
                        )
                    nc.sync.dma_start(outT_r[:, oc, qsl], outT_sb[:, oc, qsl])

    nc.compile()
    return nc


def _get_nc():
    if "nc" not in _CACHE:
        _CACHE["nc"] = _build_nc()
    return _CACHE["nc"]


def make_in_maps(query_states, Wq, bq, Wk, bk, Wv, bv, Wo, bo):
    """Host-side shard + layout prep. core c: batch c//2, head-group c%2."""
    bf = ml_dtypes.bfloat16
    x = np.asarray(query_states, np.float32)
    B = x.shape[0]
    in_maps = []
    xT_all = [np.ascontiguousarray(x[b].T).astype(bf) for b in range(B)]
    w_sl = {}
    for g in range(2):
        c0, c1 = CG * g, CG * (g + 1)
        w_sl[g] = dict(
            wqT=np.ascontiguousarray(np.asarray(Wq, np.float32)[c0:c1, :].T).astype(bf),
            wkT=np.ascontiguousarray(np.asarray(Wk, np.float32)[c0:c1, :].T).astype(bf),
            wvT=np.ascontiguousarray(np.asarray(Wv, np.float32)[c0:c1, :].T).astype(bf),
            woT=np.ascontiguousarray(np.asarray(Wo, np.float32)[:, c0:c1].T).astype(bf),
            bq=np.ascontiguousarray(np.asarray(bq, np.float32)[c0:c1]),
            bk=np.ascontiguousarray(np.asarray(bk, np.float32)[c0:c1]),
            bv=np.ascontiguousarray(np.asarray(bv, np.float32)[c0:c1]),
            bo=(np.asarray(bo, np.float32).copy() if g == 0
                else np.zeros(CIN, np.float32)),
        )
    for c in range(8):
        b, g = c // 2, c % 2
        m = {"xT": xT_all[b]}
        m.update(w_sl[g])
        in_maps.append(m)
    return in_maps


def gather_output(results):
    """Sum head-group partials per batch and transpose back to [B, S, C]."""
    B = 4
    out = np.empty((B, S, CIN), np.float32)
    for b in range(B):
        acc = (results[2 * b]["outT"].astype(np.float32)
               + results[2 * b + 1]["outT"].astype(np.float32))
        out[b] = acc.T
    return out


def kernel(query_states, Wq, bq, Wk, bk, Wv, bv, Wo, bo,
           attention_biases=None, bias_idxs=None, **_unused):
    # attention_biases/bias_idxs: bias_idxs is ones(49,49), so the gathered
    # bias is constant per head -> softmax-invariant -> no-op. Unused.
    from concourse.bass_utils import run_bass_kernel_spmd
    nc = _get_nc()
    in_maps = make_in_maps(query_states, Wq, bq, Wk, bk, Wv, bv, Wo, bo)
    res = run_bass_kernel_spmd(nc, in_maps, core_ids=list(range(8)))
    return gather_output(res.results)
